# revision 5
# baseline (speedup 1.0000x reference)
"""AnchorSegmentMixer Trainium2 kernel (8 NeuronCores, batch-sharded).

reference:
    energy[n] = mean(w[n]**2)                       # [B]
    ratio[n]  = clip(sqrt(energy[n]/max(energy[n+1 mod B], 1e-10)), 0.02, 50)
    mixtures  = w + ratio[:, None] * roll(w, -1, axis=0)
    returns (mixtures, targets=w)

Sharding: pure data parallel over the batch axis. Core c receives rows
[32c, 32c+32] (33 rows: 32 output rows + 1 circular halo row), computes all 33
row energies locally, and emits its 32 mixture rows. No collectives needed.

On-chip layout: each 160000-sample row is spread over the 128 SBUF partitions
as [128, 1250] (partition p holds samples [1250p, 1250(p+1))), and the whole
33-row shard stays resident in SBUF (161 KiB/partition) so HBM traffic is the
roofline minimum: read 33 rows + write 32 rows per core.

Structure: the 32 output rows are processed as 4 blocks of 8. Each block's
ratios only need energies of rows [8k, 8k+8], so block k's store phase
overlaps block k+1's load phase and the DMA engines stay saturated.

Engine split (measured per-[128,1250]-op costs): ACT does the 33 energy
squares (activation+accum_out, ~1.6us each) during the load phase, GpSimd the
32 ratio-scale multiplies (~1.5us), DVE the 32 adds (~1.75us) during the
store phase. vector.tensor_tensor_reduce is avoided - it crashes this runtime.
"""

import numpy as np

B = 256
S = 160000
P = 128
F = S // P            # 1250 samples per partition per row
N_CORES = 8
OUT_ROWS = B // N_CORES   # 32
ROWS = OUT_ROWS + 1       # +1 halo row
EPS = 1e-10
INV_N = 1.0 / S

BLK = 8                   # output rows per pipelined block
N_BLK = OUT_ROWS // BLK

_cache = {}


def _build_nc():
    from contextlib import ExitStack

    import concourse.bass as bass
    import concourse.tile as tile
    from concourse import bacc, mybir

    nc = bacc.Bacc("TRN2", target_bir_lowering=False, debug=False,
                   num_devices=N_CORES)
    f32 = mybir.dt.float32
    wv = nc.declare_dram_parameter("waveforms", [ROWS, S], f32, isOutput=False)
    out = nc.declare_dram_parameter("out", [OUT_ROWS, S], f32, isOutput=True)

    in_v = wv.ap().rearrange("r (p f) -> p r f", p=P)    # [128, 33, 1250]
    out_v = out.ap().rearrange("r (p f) -> p r f", p=P)  # [128, 32, 1250]

    with tile.TileContext(nc) as tc, ExitStack() as ctx:
        data_pool = ctx.enter_context(tc.tile_pool(name="data", bufs=1))
        scr_pool = ctx.enter_context(tc.tile_pool(name="scr", bufs=1))
        outp = ctx.enter_context(tc.tile_pool(name="outp", bufs=3))
        singles = ctx.enter_context(tc.tile_pool(name="singles", bufs=1))
        psum = ctx.enter_context(tc.tile_pool(name="psum", bufs=2, space="PSUM"))

        data = data_pool.tile([P, ROWS * F], f32)
        partials = singles.tile([P, ROWS], f32)       # per-partition sum(x^2)
        inv_n_col = singles.tile([P, 1], f32)         # 1/S for the mean matmul
        ones_row = singles.tile([1, P], f32)          # broadcast matmul lhsT
        e_sb = singles.tile([1, ROWS], f32)           # mean energies
        e_bc = singles.tile([P, ROWS], f32)           # energies on all partitions
        denom = singles.tile([P, OUT_ROWS], f32)
        ratio = singles.tile([P, OUT_ROWS], f32)      # clipped mix ratios
        sq_act = scr_pool.tile([P, F], f32, tag="sq_act")

        nc.vector.memset(inv_n_col[:], INV_N)
        nc.gpsimd.memset(ones_row[:], 1.0)

        def load_and_energy(r0, r1):
            # one DMA for rows [r0, r1), then per-row square+accumulate (ACT)
            nc.sync.dma_start(out=data[:, r0 * F:r1 * F], in_=in_v[:, r0:r1, :])
            for r in range(r0, r1):
                nc.scalar.activation(
                    out=sq_act[:], in_=data[:, r * F:(r + 1) * F],
                    func=mybir.ActivationFunctionType.Square,
                    accum_out=partials[:, r:r + 1],
                )

        def block_ratio(k):
            # energies for rows [8k, 8k+8] -> ratio[:, 8k:8k+8] on all parts
            lo, hi = k * BLK, k * BLK + BLK + 1
            n = hi - lo
            e_ps = psum.tile([1, n], f32, tag="e")
            nc.tensor.matmul(e_ps[:], inv_n_col[:], partials[:, lo:hi],
                             start=True, stop=True)
            nc.vector.tensor_copy(e_sb[:, lo:hi], e_ps[:])
            bc_ps = psum.tile([P, n], f32, tag="bc")
            nc.tensor.matmul(bc_ps[:], ones_row[:], e_sb[:, lo:hi],
                             start=True, stop=True)
            nc.scalar.copy(e_bc[:, lo:hi], bc_ps[:])
            rs = slice(k * BLK, k * BLK + BLK)
            nc.vector.tensor_scalar_max(denom[:, rs], e_bc[:, lo + 1:hi], EPS)
            nc.vector.reciprocal(denom[:, rs], denom[:, rs])
            nc.vector.tensor_mul(denom[:, rs], e_bc[:, lo:hi - 1], denom[:, rs])
            nc.scalar.sqrt(ratio[:, rs], denom[:, rs])
            nc.vector.tensor_scalar(
                out=ratio[:, rs], in0=ratio[:, rs], scalar1=50.0, scalar2=0.02,
                op0=mybir.AluOpType.min, op1=mybir.AluOpType.max,
            )

        def block_mix(k):
            # out[r] = w[r] + ratio[r] * w[r+1]; scale on GpSimd, add on DVE
            for r in range(k * BLK, (k + 1) * BLK):
                o = outp.tile([P, F], f32, tag="o")
                nc.gpsimd.tensor_scalar(
                    out=o[:], in0=data[:, (r + 1) * F:(r + 2) * F],
                    scalar1=ratio[:, r:r + 1], scalar2=None,
                    op0=mybir.AluOpType.mult,
                )
                nc.vector.tensor_add(o[:], o[:], data[:, r * F:(r + 1) * F])
                nc.sync.dma_start(out=out_v[:, r, :], in_=o[:])

        # software pipeline over blocks; Tile's dataflow deps do the overlap
        load_and_energy(0, BLK + 1)                 # rows 0..8 (block 0 + halo)
        for k in range(N_BLK):
            if k + 1 < N_BLK:
                lo = k * BLK + BLK + 1
                hi = min(lo + BLK, ROWS)
                load_and_energy(lo, hi)             # next block's rows
            block_ratio(k)
            block_mix(k)

    nc.compile()
    return nc


def _get_nc():
    if "nc" not in _cache:
        _cache["nc"] = _build_nc()
    return _cache["nc"]


def _shard_inputs(waveforms):
    in_maps = []
    for c in range(N_CORES):
        rows = (np.arange(c * OUT_ROWS, c * OUT_ROWS + ROWS)) % B
        in_maps.append({"waveforms": np.ascontiguousarray(waveforms[rows])})
    return in_maps


def kernel(waveforms):
    from concourse.bass_utils import run_bass_kernel_spmd

    waveforms = np.asarray(waveforms, dtype=np.float32)
    nc = _get_nc()
    in_maps = _shard_inputs(waveforms)
    res = run_bass_kernel_spmd(nc, in_maps, list(range(N_CORES)))
    mixtures = np.concatenate(
        [res.results[c]["out"] for c in range(N_CORES)], axis=0
    )
    return mixtures, waveforms


# revision 6
# speedup vs baseline: 4.6749x; 4.6749x over previous
"""AnchorSegmentMixer Trainium2 kernel (8 NeuronCores, batch-sharded).

reference:
    energy[n] = mean(w[n]**2)                       # [B]
    ratio[n]  = clip(sqrt(energy[n]/max(energy[n+1 mod B], 1e-10)), 0.02, 50)
    mixtures  = w + ratio[:, None] * roll(w, -1, axis=0)
    returns (mixtures, targets=w)

Sharding: pure data parallel over the batch axis. Core c receives rows
[32c, 32c+32] (33 rows: 32 output rows + 1 circular halo row), computes all 33
row energies locally, and emits its 32 mixture rows. No collectives needed.

On-chip layout: each 160000-sample row is spread over the 128 SBUF partitions
as [128, 1250] (partition p holds samples [1250p, 1250(p+1))), and the whole
33-row shard stays resident in SBUF (161 KiB/partition) so HBM traffic is the
roofline minimum: read 33 rows + write 32 rows per core.

Structure: the 32 output rows are processed as 4 blocks of 8. Each block's
ratios only need energies of rows [8k, 8k+8], so block k's store phase
overlaps block k+1's load phase and the DMA engines stay saturated.

Engine split (measured per-[128,1250]-op costs): ACT does the 33 energy
squares (activation+accum_out, ~1.6us each) during the load phase, GpSimd the
32 ratio-scale multiplies (~1.5us), DVE the 32 adds (~1.75us) during the
store phase. vector.tensor_tensor_reduce is avoided - it crashes this runtime.
"""

import numpy as np

B = 256
S = 160000
P = 128
F = S // P            # 1250 samples per partition per row
N_CORES = 8
OUT_ROWS = B // N_CORES   # 32
ROWS = OUT_ROWS + 1       # +1 halo row
EPS = 1e-10
INV_N = 1.0 / S

BLK = 8                   # output rows per pipelined block
N_BLK = OUT_ROWS // BLK

_cache = {}


def _build_nc():
    from contextlib import ExitStack

    import concourse.bass as bass
    import concourse.tile as tile
    from concourse import bacc, mybir

    nc = bacc.Bacc("TRN2", target_bir_lowering=False, debug=False,
                   num_devices=N_CORES)
    f32 = mybir.dt.float32
    wv = nc.declare_dram_parameter("waveforms", [ROWS, S], f32, isOutput=False)
    out = nc.declare_dram_parameter("out", [OUT_ROWS, S], f32, isOutput=True)

    in_v = wv.ap().rearrange("r (p f) -> p r f", p=P)    # [128, 33, 1250]
    out_v = out.ap().rearrange("r (p f) -> p r f", p=P)  # [128, 32, 1250]

    with tile.TileContext(nc) as tc, ExitStack() as ctx:
        data_pool = ctx.enter_context(tc.tile_pool(name="data", bufs=1))
        scr_pool = ctx.enter_context(tc.tile_pool(name="scr", bufs=1))
        outp = ctx.enter_context(tc.tile_pool(name="outp", bufs=3))
        singles = ctx.enter_context(tc.tile_pool(name="singles", bufs=1))
        psum = ctx.enter_context(tc.tile_pool(name="psum", bufs=2, space="PSUM"))

        data = data_pool.tile([P, ROWS * F], f32)
        partials = singles.tile([P, ROWS], f32)       # per-partition sum(x^2)
        inv_n_col = singles.tile([P, 1], f32)         # 1/S for the mean matmul
        ones_row = singles.tile([1, P], f32)          # broadcast matmul lhsT
        e_sb = singles.tile([1, ROWS], f32)           # mean energies
        e_bc = singles.tile([P, ROWS], f32)           # energies on all partitions
        denom = singles.tile([P, OUT_ROWS], f32)
        ratio = singles.tile([P, OUT_ROWS], f32)      # clipped mix ratios
        sq_act = scr_pool.tile([P, F], f32, tag="sq_act")

        nc.vector.memset(inv_n_col[:], INV_N)
        nc.gpsimd.memset(ones_row[:], 1.0)

        def load_and_energy(r0, r1):
            # one DMA for rows [r0, r1), then per-row square+accumulate (ACT)
            nc.sync.dma_start(out=data[:, r0 * F:r1 * F], in_=in_v[:, r0:r1, :])
            for r in range(r0, r1):
                nc.scalar.activation(
                    out=sq_act[:], in_=data[:, r * F:(r + 1) * F],
                    func=mybir.ActivationFunctionType.Square,
                    accum_out=partials[:, r:r + 1],
                )

        def block_ratio(k):
            # energies for rows [8k, 8k+8] -> ratio[:, 8k:8k+8] on all parts
            lo, hi = k * BLK, k * BLK + BLK + 1
            n = hi - lo
            e_ps = psum.tile([1, n], f32, tag="e")
            nc.tensor.matmul(e_ps[:], inv_n_col[:], partials[:, lo:hi],
                             start=True, stop=True)
            nc.vector.tensor_copy(e_sb[:, lo:hi], e_ps[:])
            bc_ps = psum.tile([P, n], f32, tag="bc")
            nc.tensor.matmul(bc_ps[:], ones_row[:], e_sb[:, lo:hi],
                             start=True, stop=True)
            nc.scalar.copy(e_bc[:, lo:hi], bc_ps[:])
            rs = slice(k * BLK, k * BLK + BLK)
            nc.vector.tensor_scalar_max(denom[:, rs], e_bc[:, lo + 1:hi], EPS)
            nc.vector.reciprocal(denom[:, rs], denom[:, rs])
            nc.vector.tensor_mul(denom[:, rs], e_bc[:, lo:hi - 1], denom[:, rs])
            nc.scalar.sqrt(ratio[:, rs], denom[:, rs])
            nc.vector.tensor_scalar(
                out=ratio[:, rs], in0=ratio[:, rs], scalar1=50.0, scalar2=0.02,
                op0=mybir.AluOpType.min, op1=mybir.AluOpType.max,
            )

        def block_mix(k):
            # out[r] = w[r] + ratio[r] * w[r+1]; scale on ACT, add on DVE
            # (NOT gpsimd: its tensor_scalar measured 19us/op vs ACT 1.6us)
            for r in range(k * BLK, (k + 1) * BLK):
                o = outp.tile([P, F], f32, tag="o")
                nc.scalar.mul(o[:], data[:, (r + 1) * F:(r + 2) * F],
                              mul=ratio[:, r:r + 1])
                nc.vector.tensor_add(o[:], o[:], data[:, r * F:(r + 1) * F])
                nc.sync.dma_start(out=out_v[:, r, :], in_=o[:])

        # software pipeline over blocks; Tile's dataflow deps do the overlap
        load_and_energy(0, BLK + 1)                 # rows 0..8 (block 0 + halo)
        for k in range(N_BLK):
            if k + 1 < N_BLK:
                lo = k * BLK + BLK + 1
                hi = min(lo + BLK, ROWS)
                load_and_energy(lo, hi)             # next block's rows
            block_ratio(k)
            block_mix(k)

    nc.compile()
    return nc


def _get_nc():
    if "nc" not in _cache:
        _cache["nc"] = _build_nc()
    return _cache["nc"]


def _shard_inputs(waveforms):
    in_maps = []
    for c in range(N_CORES):
        rows = (np.arange(c * OUT_ROWS, c * OUT_ROWS + ROWS)) % B
        in_maps.append({"waveforms": np.ascontiguousarray(waveforms[rows])})
    return in_maps


def kernel(waveforms):
    from concourse.bass_utils import run_bass_kernel_spmd

    waveforms = np.asarray(waveforms, dtype=np.float32)
    nc = _get_nc()
    in_maps = _shard_inputs(waveforms)
    res = run_bass_kernel_spmd(nc, in_maps, list(range(N_CORES)))
    mixtures = np.concatenate(
        [res.results[c]["out"] for c in range(N_CORES)], axis=0
    )
    return mixtures, waveforms


# revision 8
# speedup vs baseline: 4.7107x; 1.0077x over previous
"""AnchorSegmentMixer Trainium2 kernel (8 NeuronCores, batch-sharded).

reference:
    energy[n] = mean(w[n]**2)                       # [B]
    ratio[n]  = clip(sqrt(energy[n]/max(energy[n+1 mod B], 1e-10)), 0.02, 50)
    mixtures  = w + ratio[:, None] * roll(w, -1, axis=0)
    returns (mixtures, targets=w)

Sharding: pure data parallel over the batch axis. Core c receives rows
[32c, 32c+32] (33 rows: 32 output rows + 1 circular halo row), computes all 33
row energies locally, and emits its 32 mixture rows. No collectives needed.

On-chip layout: each 160000-sample row is spread over the 128 SBUF partitions
as [128, 1250] (partition p holds samples [1250p, 1250(p+1))), and the whole
33-row shard stays resident in SBUF (161 KiB/partition) so HBM traffic is the
roofline minimum: read 33 rows + write 32 rows per core.

Structure: the 32 output rows are processed as 4 blocks of 8. Each block's
ratios only need energies of rows [8k, 8k+8], so block k's store phase
overlaps block k+1's load phase and the DMA engines stay saturated.

Engine split (measured per-[128,1250]-op costs): ACT does the 33 energy
squares (activation+accum_out, ~1.6us each) during the load phase, GpSimd the
32 ratio-scale multiplies (~1.5us), DVE the 32 adds (~1.75us) during the
store phase. vector.tensor_tensor_reduce is avoided - it crashes this runtime.
"""

import numpy as np

B = 256
S = 160000
P = 128
F = S // P            # 1250 samples per partition per row
N_CORES = 8
OUT_ROWS = B // N_CORES   # 32
ROWS = OUT_ROWS + 1       # +1 halo row
EPS = 1e-10
INV_N = 1.0 / S

BLK = 8                   # output rows per pipelined block
N_BLK = OUT_ROWS // BLK

_cache = {}


def _build_nc():
    from contextlib import ExitStack

    import concourse.bass as bass
    import concourse.tile as tile
    from concourse import bacc, mybir

    nc = bacc.Bacc("TRN2", target_bir_lowering=False, debug=False,
                   num_devices=N_CORES)
    f32 = mybir.dt.float32
    wv = nc.declare_dram_parameter("waveforms", [ROWS, S], f32, isOutput=False)
    out = nc.declare_dram_parameter("out", [OUT_ROWS, S], f32, isOutput=True)

    in_v = wv.ap().rearrange("r (p f) -> p r f", p=P)    # [128, 33, 1250]
    out_v = out.ap().rearrange("r (p f) -> p r f", p=P)  # [128, 32, 1250]

    with tile.TileContext(nc) as tc, ExitStack() as ctx:
        data_pool = ctx.enter_context(tc.tile_pool(name="data", bufs=1))
        scr_pool = ctx.enter_context(tc.tile_pool(name="scr", bufs=1))
        outp = ctx.enter_context(tc.tile_pool(name="outp", bufs=3))
        singles = ctx.enter_context(tc.tile_pool(name="singles", bufs=1))
        psum = ctx.enter_context(tc.tile_pool(name="psum", bufs=2, space="PSUM"))

        data = data_pool.tile([P, ROWS * F], f32)
        partials = singles.tile([P, ROWS], f32)       # per-partition sum(x^2)
        inv_n_col = singles.tile([P, 1], f32)         # 1/S for the mean matmul
        ones_row = singles.tile([1, P], f32)          # broadcast matmul lhsT
        e_sb = singles.tile([1, ROWS], f32)           # mean energies
        e_bc = singles.tile([P, ROWS], f32)           # energies on all partitions
        denom = singles.tile([P, OUT_ROWS], f32)
        ratio = singles.tile([P, OUT_ROWS], f32)      # clipped mix ratios
        sq_act = scr_pool.tile([P, F], f32, tag="sq_act")

        nc.vector.memset(inv_n_col[:], INV_N)
        nc.gpsimd.memset(ones_row[:], 1.0)

        def load_and_energy(r0, r1):
            # one DMA for rows [r0, r1), then per-row square+accumulate (ACT)
            nc.sync.dma_start(out=data[:, r0 * F:r1 * F], in_=in_v[:, r0:r1, :])
            for r in range(r0, r1):
                nc.scalar.activation(
                    out=sq_act[:], in_=data[:, r * F:(r + 1) * F],
                    func=mybir.ActivationFunctionType.Square,
                    accum_out=partials[:, r:r + 1],
                )

        def block_ratio(k):
            # energies for rows [8k, 8k+8] -> ratio[:, 8k:8k+8] on all parts
            lo, hi = k * BLK, k * BLK + BLK + 1
            n = hi - lo
            e_ps = psum.tile([1, n], f32, tag="e")
            nc.tensor.matmul(e_ps[:], inv_n_col[:], partials[:, lo:hi],
                             start=True, stop=True)
            nc.vector.tensor_copy(e_sb[:, lo:hi], e_ps[:])
            bc_ps = psum.tile([P, n], f32, tag="bc")
            nc.tensor.matmul(bc_ps[:], ones_row[:], e_sb[:, lo:hi],
                             start=True, stop=True)
            nc.vector.tensor_copy(e_bc[:, lo:hi], bc_ps[:])
            rs = slice(k * BLK, k * BLK + BLK)
            nc.vector.tensor_scalar_max(denom[:, rs], e_bc[:, lo + 1:hi], EPS)
            nc.vector.reciprocal(denom[:, rs], denom[:, rs])
            nc.vector.tensor_mul(denom[:, rs], e_bc[:, lo:hi - 1], denom[:, rs])
            nc.scalar.sqrt(ratio[:, rs], denom[:, rs])
            nc.vector.tensor_scalar(
                out=ratio[:, rs], in0=ratio[:, rs], scalar1=50.0, scalar2=0.02,
                op0=mybir.AluOpType.min, op1=mybir.AluOpType.max,
            )

        def block_mix(k):
            # out[r] = w[r] + ratio[r] * w[r+1]; adds on DVE, scales split
            # ACT/DVE to balance engine busy (ACT also owns the squares).
            # (NOT gpsimd: its tensor_scalar measured 19us/op vs ACT 1.6us)
            for r in range(k * BLK, (k + 1) * BLK):
                o = outp.tile([P, F], f32, tag="o")
                nxt = data[:, (r + 1) * F:(r + 2) * F]
                if r % BLK < 3:
                    nc.vector.tensor_scalar_mul(o[:], nxt, ratio[:, r:r + 1])
                else:
                    nc.scalar.mul(o[:], nxt, mul=ratio[:, r:r + 1])
                nc.vector.tensor_add(o[:], o[:], data[:, r * F:(r + 1) * F])
                nc.sync.dma_start(out=out_v[:, r, :], in_=o[:])

        # software pipeline over blocks; Tile's dataflow deps do the overlap
        load_and_energy(0, BLK + 1)                 # rows 0..8 (block 0 + halo)
        for k in range(N_BLK):
            if k + 1 < N_BLK:
                lo = k * BLK + BLK + 1
                hi = min(lo + BLK, ROWS)
                load_and_energy(lo, hi)             # next block's rows
            block_ratio(k)
            block_mix(k)

    nc.compile()
    return nc


def _get_nc():
    if "nc" not in _cache:
        _cache["nc"] = _build_nc()
    return _cache["nc"]


def _shard_inputs(waveforms):
    in_maps = []
    for c in range(N_CORES):
        rows = (np.arange(c * OUT_ROWS, c * OUT_ROWS + ROWS)) % B
        in_maps.append({"waveforms": np.ascontiguousarray(waveforms[rows])})
    return in_maps


def kernel(waveforms):
    from concourse.bass_utils import run_bass_kernel_spmd

    waveforms = np.asarray(waveforms, dtype=np.float32)
    nc = _get_nc()
    in_maps = _shard_inputs(waveforms)
    res = run_bass_kernel_spmd(nc, in_maps, list(range(N_CORES)))
    mixtures = np.concatenate(
        [res.results[c]["out"] for c in range(N_CORES)], axis=0
    )
    return mixtures, waveforms


# revision 10
# speedup vs baseline: 4.9705x; 1.0551x over previous
"""AnchorSegmentMixer Trainium2 kernel (8 NeuronCores, batch-sharded).

reference:
    energy[n] = mean(w[n]**2)                       # [B]
    ratio[n]  = clip(sqrt(energy[n]/max(energy[n+1 mod B], 1e-10)), 0.02, 50)
    mixtures  = w + ratio[:, None] * roll(w, -1, axis=0)
    returns (mixtures, targets=w)

Sharding: pure data parallel over the batch axis. Core c receives rows
[32c, 32c+32] (33 rows: 32 output rows + 1 circular halo row), computes all 33
row energies locally, and emits its 32 mixture rows. No collectives needed.

On-chip layout: each 160000-sample row is spread over the 128 SBUF partitions
as [128, 1250] (partition p holds samples [1250p, 1250(p+1))), and the whole
33-row shard stays resident in SBUF (161 KiB/partition) so HBM traffic is the
roofline minimum: read 33 rows + write 32 rows per core.

Structure: the 32 output rows are processed as 4 blocks of 8. Each block's
ratios only need energies of rows [8k, 8k+8], so block k's store phase
overlaps block k+1's load phase and the DMA engines stay saturated.

Engine split (measured per-[128,1250]-op costs): ACT does the 33 energy
squares (activation+accum_out, ~1.6us each) during the load phase, GpSimd the
32 ratio-scale multiplies (~1.5us), DVE the 32 adds (~1.75us) during the
store phase. vector.tensor_tensor_reduce is avoided - it crashes this runtime.
"""

import numpy as np

B = 256
S = 160000
P = 128
F = S // P            # 1250 samples per partition per row
N_CORES = 8
OUT_ROWS = B // N_CORES   # 32
ROWS = OUT_ROWS + 1       # +1 halo row
EPS = 1e-10
INV_N = 1.0 / S

# pipelined block sizes: small first block (fast ramp to the first output
# DMAs), small last block (short drain tail), 8-row blocks in the middle
BLOCK_SIZES = (4, 8, 8, 8, 4)
assert sum(BLOCK_SIZES) == OUT_ROWS

_cache = {}


def _build_nc():
    from contextlib import ExitStack

    import concourse.bass as bass
    import concourse.tile as tile
    from concourse import bacc, mybir

    nc = bacc.Bacc("TRN2", target_bir_lowering=False, debug=False,
                   num_devices=N_CORES)
    f32 = mybir.dt.float32
    wv = nc.declare_dram_parameter("waveforms", [ROWS, S], f32, isOutput=False)
    out = nc.declare_dram_parameter("out", [OUT_ROWS, S], f32, isOutput=True)

    in_v = wv.ap().rearrange("r (p f) -> p r f", p=P)    # [128, 33, 1250]
    out_v = out.ap().rearrange("r (p f) -> p r f", p=P)  # [128, 32, 1250]

    with tile.TileContext(nc) as tc, ExitStack() as ctx:
        data_pool = ctx.enter_context(tc.tile_pool(name="data", bufs=1))
        scr_pool = ctx.enter_context(tc.tile_pool(name="scr", bufs=1))
        outp = ctx.enter_context(tc.tile_pool(name="outp", bufs=3))
        singles = ctx.enter_context(tc.tile_pool(name="singles", bufs=1))
        psum = ctx.enter_context(tc.tile_pool(name="psum", bufs=2, space="PSUM"))

        data = data_pool.tile([P, ROWS * F], f32)
        partials = singles.tile([P, ROWS], f32)       # per-partition sum(x^2)
        inv_n_col = singles.tile([P, 1], f32)         # 1/S for the mean matmul
        ones_row = singles.tile([1, P], f32)          # broadcast matmul lhsT
        e_sb = singles.tile([1, ROWS], f32)           # mean energies
        e_bc = singles.tile([P, ROWS], f32)           # energies on all partitions
        denom = singles.tile([P, OUT_ROWS], f32)
        ratio = singles.tile([P, OUT_ROWS], f32)      # clipped mix ratios
        sq_act = scr_pool.tile([P, F], f32, tag="sq_act")

        nc.vector.memset(inv_n_col[:], INV_N)
        nc.gpsimd.memset(ones_row[:], 1.0)

        def load_rows(r0, r1, split=1):
            # in-DMAs ride GpSimd/SWDGE: gpsimd is otherwise idle, so loads
            # are never queued behind out-DMAs on Sync's in-order stream
            step = max(1, (r1 - r0 + split - 1) // split)
            for g in range(r0, r1, step):
                ge = min(g + step, r1)
                nc.gpsimd.dma_start(out=data[:, g * F:ge * F],
                                    in_=in_v[:, g:ge, :])

        def square(r):
            nc.scalar.activation(
                out=sq_act[:], in_=data[:, r * F:(r + 1) * F],
                func=mybir.ActivationFunctionType.Square,
                accum_out=partials[:, r:r + 1],
            )

        def block_ratio(lo, hi):
            # energies for rows [lo, hi] -> ratio[:, lo:hi] on all partitions
            n = hi - lo + 1
            e_ps = psum.tile([1, n], f32, tag="e")
            nc.tensor.matmul(e_ps[:], inv_n_col[:], partials[:, lo:hi + 1],
                             start=True, stop=True)
            nc.vector.tensor_copy(e_sb[:, lo:hi + 1], e_ps[:])
            bc_ps = psum.tile([P, n], f32, tag="bc")
            nc.tensor.matmul(bc_ps[:], ones_row[:], e_sb[:, lo:hi + 1],
                             start=True, stop=True)
            nc.vector.tensor_copy(e_bc[:, lo:hi + 1], bc_ps[:])
            rs = slice(lo, hi)
            nc.vector.tensor_scalar_max(denom[:, rs], e_bc[:, lo + 1:hi + 1], EPS)
            nc.vector.reciprocal(denom[:, rs], denom[:, rs])
            nc.vector.tensor_mul(denom[:, rs], e_bc[:, lo:hi], denom[:, rs])
            nc.scalar.sqrt(ratio[:, rs], denom[:, rs])
            nc.vector.tensor_scalar(
                out=ratio[:, rs], in0=ratio[:, rs], scalar1=50.0, scalar2=0.02,
                op0=mybir.AluOpType.min, op1=mybir.AluOpType.max,
            )

        def mix_row(r, on_dve):
            # out[r] = w[r] + ratio[r] * w[r+1]; adds on DVE, scales split
            # ACT/DVE to balance engine busy (ACT also owns the squares).
            # (NOT gpsimd: its tensor_scalar measured 19us/op vs ACT 1.6us)
            o = outp.tile([P, F], f32, tag="o")
            nxt = data[:, (r + 1) * F:(r + 2) * F]
            if on_dve:
                nc.vector.tensor_scalar_mul(o[:], nxt, ratio[:, r:r + 1])
            else:
                nc.scalar.mul(o[:], nxt, mul=ratio[:, r:r + 1])
            nc.vector.tensor_add(o[:], o[:], data[:, r * F:(r + 1) * F])
            nc.sync.dma_start(out=out_v[:, r, :], in_=o[:])

        # Software pipeline over blocks. Emission order is chosen so that on
        # ACT the current block's scale-muls come BEFORE the next block's
        # squares (except the one halo square ratio(k) needs) - otherwise the
        # store phase stalls ~10us per block behind the square batch.
        starts = [sum(BLOCK_SIZES[:i]) for i in range(len(BLOCK_SIZES))]
        load_rows(0, starts[1] + 1, split=starts[1] + 1)  # block 0 + halo,
        for r in range(starts[1] + 1):                    # per-row for ramp
            square(r)
        for k, sz in enumerate(BLOCK_SIZES):
            lo = starts[k]
            nxt_lo = lo + sz
            last = k + 1 == len(BLOCK_SIZES)
            if not last:
                nxt_sz = BLOCK_SIZES[k + 1]
                load_rows(nxt_lo + 1, nxt_lo + nxt_sz + 1)
            block_ratio(lo, nxt_lo)
            for i, r in enumerate(range(lo, nxt_lo)):
                mix_row(r, on_dve=(i % 8 < 3))
            if not last:
                for r in range(nxt_lo + 1, nxt_lo + nxt_sz + 1):
                    square(r)

    nc.compile()
    return nc


def _get_nc():
    if "nc" not in _cache:
        _cache["nc"] = _build_nc()
    return _cache["nc"]


def _shard_inputs(waveforms):
    in_maps = []
    for c in range(N_CORES):
        rows = (np.arange(c * OUT_ROWS, c * OUT_ROWS + ROWS)) % B
        in_maps.append({"waveforms": np.ascontiguousarray(waveforms[rows])})
    return in_maps


def kernel(waveforms):
    from concourse.bass_utils import run_bass_kernel_spmd

    waveforms = np.asarray(waveforms, dtype=np.float32)
    nc = _get_nc()
    in_maps = _shard_inputs(waveforms)
    res = run_bass_kernel_spmd(nc, in_maps, list(range(N_CORES)))
    mixtures = np.concatenate(
        [res.results[c]["out"] for c in range(N_CORES)], axis=0
    )
    return mixtures, waveforms
